# revision 12
# baseline (speedup 1.0000x reference)
"""Trainium2 Bass kernel for nn_DiTeBlock (GNN message passing + row-parallel
attention + SwiGLU FFNs), sharded across 8 NeuronCores.

Sharding: nodes split into 8 contiguous blocks of 128 (core c owns rows
[128c, 128c+128)); edges are routed to the core owning their src node
(host-side stable sort), padded to a common capacity E_cap. Z and the
attention score matrix shard along the query-node dim; K/V come from an
AllGather of per-core aggregated node features. Weights are replicated.

Self-contained: hardcodes N=1024, H=256, NH=8, DV=128, FF=1024, 8 cores.
"""

import math
import numpy as np

import concourse.bass as bass
import concourse.mybir as mybir
import concourse.tile as tile
from concourse import bacc
from concourse.bass_utils import run_bass_kernel_spmd
from concourse.masks import make_identity

P = 128
N = 1024
H = 256
NH = 8
DHEAD = H // NH          # 32
DV = 128
FF = 1024
NC = 8
NLOC = N // NC           # 128
KH = H // P              # 2
NMT = N // P             # 8
F32 = mybir.dt.float32
F32R = mybir.dt.float32r
I32 = mybir.dt.int32
AF = mybir.ActivationFunctionType
ALU = mybir.AluOpType
AX = mybir.AxisListType
NEGBIG = -1e9
SQD = math.sqrt(DHEAD)
ISCALE = 1.0 / SQD


def _brows(t, n_rows, width, offset=0):
    """DRAM AP replicating a 1-D tensor slice across n_rows partitions."""
    return bass.AP(tensor=t, offset=offset, ap=[[0, n_rows], [1, width]])


def build_kernel(e_cap, debug=False):
    nt_e = e_cap // P
    assert e_cap % 512 == 0

    nc = bacc.Bacc("TRN2", target_bir_lowering=False, debug=False, num_devices=NC)

    def din(name, shape, dt=F32):
        return nc.dram_tensor(name, shape, dt, kind="ExternalInput")

    x_full = din("x_full", [N, H])
    th_full = din("th_full", [N, H])
    x_own = din("x_own", [NLOC, H])
    th_own = din("th_own", [NLOC, H])
    batch_f = din("batch_f", [N])
    batch_own = din("batch_own", [NLOC, 1])
    row_ids = din("row_ids", [NLOC, 1])
    z_loc = din("z_loc", [NLOC, N, H])
    ea_loc = din("ea_loc", [e_cap, H])
    te_loc = din("te_loc", [e_cap, H])
    dist_loc = din("dist_loc", [e_cap, DV])
    src_g = din("src_g", [e_cap, 1], I32)
    tgt_g = din("tgt_g", [e_cap, 1], I32)
    a_t = din("a_t", [e_cap, NLOC], F32R)

    w_adaln = din("w_adaln", [H, 6 * H], F32R)
    b_adaln = din("b_adaln", [6 * H])
    w_adaln_e = din("w_adaln_e", [H, 6 * H], F32R)
    b_adaln_e = din("b_adaln_e", [6 * H])
    w_fe1 = din("w_fe1", [3 * H + DV, H], F32R)
    b_fe1 = din("b_fe1", [H])
    w_fe2 = din("w_fe2", [H, H], F32R)
    b_fe2 = din("b_fe2", [H])
    w_qkv = din("w_qkv", [H, 3 * H], F32R)
    w_out = din("w_out", [H, H], F32R)
    w_pb = din("w_pb", [H, 1])
    w_e0 = din("w_e0", [H, H], F32R)
    w_e1 = din("w_e1", [H + DV, H], F32R)
    g_ffn = din("g_ffn", [H])
    bt_ffn = din("bt_ffn", [H])
    w_ffn1 = din("w_ffn1", [H, 2 * FF], F32R)
    w_ffn2 = din("w_ffn2", [FF, H], F32R)
    g_ffn_e = din("g_ffn_e", [H])
    bt_ffn_e = din("bt_ffn_e", [H])
    w_ffn_e1 = din("w_ffn_e1", [H, 2 * FF], F32R)
    w_ffn_e2 = din("w_ffn_e2", [FF, H], F32R)

    x3_out = nc.dram_tensor("x3_out", [NLOC, H], F32, kind="ExternalOutput")
    edge_out = nc.dram_tensor("edge_out", [e_cap, H], F32, kind="ExternalOutput")
    if debug:
        dbg_xagg = nc.dram_tensor("dbg_xagg", [N, H], F32, kind="ExternalOutput")
        dbg_y = nc.dram_tensor("dbg_y", [NLOC, H], F32, kind="ExternalOutput")
        dbg_zbias = nc.dram_tensor("dbg_zbias", [NLOC, N], F32, kind="ExternalOutput")

    with tile.TileContext(nc) as tc:
        with (
            tc.tile_pool(name="cst", bufs=1) as cst,
            tc.tile_pool(name="wts", bufs=1) as wts,
            tc.tile_pool(name="keep", bufs=1) as keep,
            tc.tile_pool(name="sb", bufs=2) as sb,
            tc.tile_pool(name="lnp", bufs=4) as lnp,
            tc.tile_pool(name="zp", bufs=2) as zp,
            tc.tile_pool(name="big1", bufs=1) as big1,
            tc.tile_pool(name="ps_t", bufs=3, space="PSUM") as ps_t,
            tc.tile_pool(name="ps_m", bufs=2, space="PSUM") as ps_m,
            tc.tile_pool(name="ps_w", bufs=2, space="PSUM") as ps_w,
            tc.tile_pool(name="ps_av", bufs=1, space="PSUM") as ps_av,
            tc.tile_pool(name="dram", bufs=1, space="DRAM") as dram,
        ):
            # ---------------- constants ----------------
            ident = cst.tile([P, P], F32)
            make_identity(nc, ident[:])
            iota = cst.tile([P, N], F32)
            nc.gpsimd.iota(iota[:], pattern=[[1, N]], base=0, channel_multiplier=0,
                           allow_small_or_imprecise_dtypes=True)
            batch_bc = cst.tile([P, N], F32)
            nc.sync.dma_start(batch_bc[:], _brows(batch_f, P, N))
            w_pb_bc = cst.tile([P, H], F32)
            nc.sync.dma_start(w_pb_bc[:], bass.AP(tensor=w_pb, offset=0,
                                                  ap=[[0, P], [1, H]]))
            rowids = cst.tile([P, 1], F32)
            nc.sync.dma_start(rowids[:], row_ids[:, :])
            batch_own_t = cst.tile([P, 1], F32)
            nc.sync.dma_start(batch_own_t[:], batch_own[:, :])
            eps_t = cst.tile([P, 1], F32)
            nc.vector.memset(eps_t[:], 1e-6)

            def bcast1d(pool, t, width, offset, tag, plus1=None):
                bt = pool.tile([P, width], F32, tag=tag, bufs=1)
                nc.sync.dma_start(bt[:], _brows(t, P, width, offset))
                if plus1 is not None:
                    nc.scalar.add(bt[:, plus1[0]:plus1[1]],
                                  bt[:, plus1[0]:plus1[1]], 1.0)
                return bt

            # node/edge adaLN bias chunks share slots (sequential phases)
            b_n1 = bcast1d(sb, b_adaln, 512, 0, "b_half", plus1=(H, 2 * H))
            b_n2 = bcast1d(sb, b_adaln, 1024, 512, "b_full", plus1=(2 * H, 3 * H))
            b_fe1_bc = bcast1d(cst, b_fe1, H, 0, "bfe1")
            b_fe2_bc = bcast1d(cst, b_fe2, H, 0, "bfe2")
            g_ffn_bc = bcast1d(cst, g_ffn, H, 0, "gffn")
            bt_ffn_bc = bcast1d(cst, bt_ffn, H, 0, "btffn")
            g_ffn_e_bc = bcast1d(cst, g_ffn_e, H, 0, "gffne")
            bt_ffn_e_bc = bcast1d(cst, bt_ffn_e, H, 0, "btffne")

            # ---------------- weights ----------------
            def wload(pool, t, rows, cols, tag):
                wt = pool.tile([P, rows // P, cols], F32R, tag=tag)
                nc.sync.dma_start(wt[:], t.ap().rearrange("(ko ki) n -> ki ko n",
                                                          ki=P))
                return wt

            # big1 slot timeline: W_adaLN (N1/N2) -> W_ffn1 (N3) -> W_ffn_e1 (E2)
            w_adaln_t = wload(big1, w_adaln, H, 6 * H, "wbig1")
            w_fe1_t = wload(wts, w_fe1, 896, H, "wbig2")
            w_fe2_t = wload(wts, w_fe2, H, H, "wfe2")
            w_qkv_t = wload(wts, w_qkv, H, 3 * H, "wqkv")
            w_out_t = wload(wts, w_out, H, H, "wout")
            w_e0_t = wload(wts, w_e0, H, H, "we0")
            w_e1_t = wload(wts, w_e1, 384, H, "we1")
            # edge adaLN chunk for E1 (cols 0:512)
            w_ae1_t = wts.tile([P, KH, 512], F32R, tag="wae")
            nc.sync.dma_start(
                w_ae1_t[:],
                w_adaln_e.ap()[:, 0:512].rearrange("(ko ki) n -> ki ko n", ki=P))

            # DRAM scratch
            u_s_d = dram.tile([N, H], F32)
            u_t_d = dram.tile([N, H], F32)
            v_d = dram.tile([N, H], F32)
            ag_in = dram.tile([NLOC, H], F32)
            ag_out = dram.tile([N, H], F32, addr_space="Shared")
            yag_in = dram.tile([NLOC, H], F32)
            yg = dram.tile([N, H], F32, addr_space="Shared")

            def transpose_to(dest_ap, src_ap, w=P):
                pt = ps_t.tile([P, P], F32, tag="tp", space="PSUM")
                nc.tensor.transpose(pt[:w, :], src_ap, ident[:])
                nc.vector.tensor_copy(dest_ap, pt[:w, :])

            def ln_rows(ap_in, out_ap):
                stats = lnp.tile([P, nc.vector.BN_STATS_DIM], F32, tag="lnS")
                nc.vector.bn_stats(stats[:], ap_in)
                mv = lnp.tile([P, nc.vector.BN_AGGR_DIM], F32, tag="lnM")
                nc.vector.bn_aggr(mv[:], stats[:])
                std = lnp.tile([P, 1], F32, tag="lnD")
                nc.scalar.activation(std[:], mv[:, 1:2], AF.Sqrt, bias=eps_t[:],
                                     scale=1.0)
                rstd = lnp.tile([P, 1], F32, tag="lnR")
                nc.vector.reciprocal(rstd[:], std[:])
                nc.vector.tensor_scalar(out_ap, ap_in, mv[:, 0:1], rstd[:, 0:1],
                                        op0=ALU.subtract, op1=ALU.mult)

            # ---------------- Z streaming (interleaved) ----------------
            zbias = keep.tile([P, N], F32)
            n_super = N // 4
            z_cursor = [0]

            def z_chunk(count):
                for _ in range(count):
                    j0 = z_cursor[0]
                    if j0 >= n_super:
                        return
                    z_cursor[0] += 1
                    zt = zp.tile([P, 4, H], F32, tag="zt")
                    nc.sync.dma_start(zt[:], z_loc[:, j0 * 4:(j0 + 1) * 4, :])
                    nc.scalar.activation(zt[:], zt[:], AF.Silu)
                    wb = bass.AP(tensor=w_pb_bc.tensor, offset=w_pb_bc[:].offset,
                                 ap=[w_pb_bc[:].ap[0], [0, 4], [1, H]])
                    nc.gpsimd.tensor_tensor(zt[:], zt[:], wb, op=ALU.mult)
                    nc.vector.reduce_sum(zbias[:, j0 * 4:(j0 + 1) * 4], zt[:],
                                         axis=AX.X)

            # ================= N1: x_n -> u_s, u_t (all nodes) =============
            for m in range(NMT):
                r0 = m * P
                th = sb.tile([P, H], F32, tag="t_in")
                nc.sync.dma_start(th[:], th_full[r0:r0 + P, :])
                st = sb.tile([P, H], F32, tag="t_silu")
                nc.scalar.activation(st[:], th[:], AF.Silu)
                stT = sb.tile([P, KH, P], F32R, tag="t_siluT")
                for ko in range(KH):
                    transpose_to(stT[:, ko], st[:, ko * P:(ko + 1) * P])
                mh_ps = ps_w.tile([P, 512], F32, tag="mm512", space="PSUM")
                for ko in range(KH):
                    nc.tensor.matmul(mh_ps[:], stT[:, ko], w_adaln_t[:, ko, 0:512],
                                     start=(ko == 0), stop=(ko == KH - 1))
                sc_sh = sb.tile([P, 512], F32, tag="half512")
                nc.vector.tensor_add(sc_sh[:], mh_ps[:], b_n1[:])

                xt = sb.tile([P, H], F32, tag="x_in")
                nc.sync.dma_start(xt[:], x_full[r0:r0 + P, :])
                x_ln = sb.tile([P, H], F32, tag="x_ln")
                ln_rows(xt[:], x_ln[:])
                xn = sb.tile([P, H], F32, tag="x_n")
                nc.vector.tensor_mul(xn[:], x_ln[:], sc_sh[:, H:2 * H])
                nc.vector.tensor_add(xn[:], xn[:], sc_sh[:, 0:H])
                xnT = sb.tile([P, KH, P], F32R, tag="x_nT")
                for ko in range(KH):
                    transpose_to(xnT[:, ko], xn[:, ko * P:(ko + 1) * P])
                us_ps = ps_m.tile([P, H], F32, tag="mm256", space="PSUM")
                for ko in range(KH):
                    nc.tensor.matmul(us_ps[:], xnT[:, ko], w_fe1_t[:, ko],
                                     start=(ko == 0), stop=(ko == KH - 1))
                ut_ps = ps_m.tile([P, H], F32, tag="mm256", space="PSUM")
                for ko in range(KH):
                    nc.tensor.matmul(ut_ps[:], xnT[:, ko], w_fe1_t[:, 2 + ko],
                                     start=(ko == 0), stop=(ko == KH - 1))
                us = sb.tile([P, H], F32, tag="u_out")
                nc.vector.tensor_add(us[:], us_ps[:], b_fe1_bc[:])
                ut = sb.tile([P, H], F32, tag="u_out")
                nc.scalar.copy(ut[:], ut_ps[:])
                nc.sync.dma_start(u_s_d[r0:r0 + P, :], us[:])
                nc.sync.dma_start(u_t_d[r0:r0 + P, :], ut[:])
                z_chunk(8)

            # ================= N2: own adaLN chunks ========================
            th_o = sb.tile([P, H], F32, tag="t_in")
            nc.sync.dma_start(th_o[:], th_own[:, :])
            st_o = sb.tile([P, H], F32, tag="t_silu")
            nc.scalar.activation(st_o[:], th_o[:], AF.Silu)
            stT_o = sb.tile([P, KH, P], F32R, tag="t_siluT")
            for ko in range(KH):
                transpose_to(stT_o[:, ko], st_o[:, ko * P:(ko + 1) * P])
            own4 = keep.tile([P, 4, H], F32)   # [g_msa, sh_mlp, sc_mlp+1, g_mlp]
            for half in range(2):
                hp = ps_w.tile([P, 512], F32, tag="mm512", space="PSUM")
                for ko in range(KH):
                    nc.tensor.matmul(
                        hp[:], stT_o[:, ko],
                        w_adaln_t[:, ko, 512 + half * 512:1024 + half * 512],
                        start=(ko == 0), stop=(ko == KH - 1))
                nc.vector.tensor_add(
                    own4[:, 2 * half:2 * half + 2].rearrange("p a b -> p (a b)"),
                    hp[:], b_n2[:, half * 512:(half + 1) * 512])

            # ================= E1 (+ Z interleave) =========================
            xagg = keep.tile([P, H], F32)
            nc.vector.memset(xagg[:], 0.0)
            b_e1 = bcast1d(sb, b_adaln_e, 512, 0, "b_half", plus1=(H, 2 * H))
            for it in range(nt_e):
                e0 = it * P
                te = sb.tile([P, H], F32, tag="t_in")
                nc.sync.dma_start(te[:], te_loc[e0:e0 + P, :])
                ste = sb.tile([P, H], F32, tag="t_silu")
                nc.scalar.activation(ste[:], te[:], AF.Silu)
                steT = sb.tile([P, KH, P], F32R, tag="t_siluT")
                for ko in range(KH):
                    transpose_to(steT[:, ko], ste[:, ko * P:(ko + 1) * P])
                me_ps = ps_w.tile([P, 512], F32, tag="mm512", space="PSUM")
                for ko in range(KH):
                    nc.tensor.matmul(me_ps[:], steT[:, ko], w_ae1_t[:, ko],
                                     start=(ko == 0), stop=(ko == KH - 1))
                me1 = sb.tile([P, 512], F32, tag="half512")
                nc.vector.tensor_add(me1[:], me_ps[:], b_e1[:])

                ea = sb.tile([P, H], F32, tag="x_in")
                nc.sync.dma_start(ea[:], ea_loc[e0:e0 + P, :])
                ea_ln = sb.tile([P, H], F32, tag="x_ln")
                ln_rows(ea[:], ea_ln[:])
                en = sb.tile([P, H], F32, tag="x_n")
                nc.vector.tensor_mul(en[:], ea_ln[:], me1[:, H:2 * H])
                nc.vector.tensor_add(en[:], en[:], me1[:, 0:H])
                enT = sb.tile([P, KH, P], F32R, tag="x_nT")
                for ko in range(KH):
                    transpose_to(enT[:, ko], en[:, ko * P:(ko + 1) * P])
                dt_ = sb.tile([P, DV], F32, tag="d_in")
                nc.sync.dma_start(dt_[:], dist_loc[e0:e0 + P, :])
                dT = sb.tile([P, P], F32R, tag="d_T")
                transpose_to(dT[:], dt_[:])

                sidx = sb.tile([P, 1], I32, tag="si")
                nc.sync.dma_start(sidx[:], src_g[e0:e0 + P, :])
                tidx = sb.tile([P, 1], I32, tag="ti")
                nc.sync.dma_start(tidx[:], tgt_g[e0:e0 + P, :])
                usg = sb.tile([P, H], F32, tag="g_us")
                nc.gpsimd.indirect_dma_start(
                    out=usg[:], out_offset=None, in_=u_s_d[:, :],
                    in_offset=bass.IndirectOffsetOnAxis(ap=sidx[:, 0:1], axis=0))
                utg = sb.tile([P, H], F32, tag="g_ut")
                nc.gpsimd.indirect_dma_start(
                    out=utg[:], out_offset=None, in_=u_t_d[:, :],
                    in_offset=bass.IndirectOffsetOnAxis(ap=tidx[:, 0:1], axis=0))

                hp_ps = ps_m.tile([P, H], F32, tag="mm256", space="PSUM")
                for ko in range(KH):
                    nc.tensor.matmul(hp_ps[:], enT[:, ko], w_fe1_t[:, 4 + ko],
                                     start=(ko == 0), stop=False)
                nc.tensor.matmul(hp_ps[:], dT[:], w_fe1_t[:, 6],
                                 start=False, stop=True)
                hsum = sb.tile([P, H], F32, tag="h_sum")
                nc.vector.tensor_add(hsum[:], hp_ps[:], usg[:])
                nc.gpsimd.tensor_tensor(hsum[:], hsum[:], utg[:], op=ALU.add)
                hfe = sb.tile([P, H], F32, tag="h_fe")
                nc.scalar.activation(hfe[:], hsum[:], AF.Silu)
                hfT = sb.tile([P, KH, P], F32R, tag="h_feT")
                for ko in range(KH):
                    transpose_to(hfT[:, ko], hfe[:, ko * P:(ko + 1) * P])
                ms_ps = ps_m.tile([P, H], F32, tag="mm256", space="PSUM")
                for ko in range(KH):
                    nc.tensor.matmul(ms_ps[:], hfT[:, ko], w_fe2_t[:, ko],
                                     start=(ko == 0), stop=(ko == KH - 1))
                msgs = sb.tile([P, H], F32R, tag="msgs")
                nc.vector.tensor_add(msgs[:], ms_ps[:], b_fe2_bc[:])
                at_t = sb.tile([P, NLOC], F32R, tag="a_t")
                nc.sync.dma_start(at_t[:], a_t[e0:e0 + P, :])
                xa_ps = ps_m.tile([P, H], F32, tag="mm256", space="PSUM")
                nc.tensor.matmul(xa_ps[:], at_t[:], msgs[:], start=True, stop=True)
                nc.vector.tensor_add(xagg[:], xagg[:], xa_ps[:])
                z_chunk(6 if it % 2 == 0 else 5)

            z_chunk(n_super)  # drain

            # ================= AllGather x_agg =============================
            nc.sync.dma_start(ag_in[:], xagg[:])
            nc.gpsimd.collective_compute(
                "AllGather", ALU.bypass, replica_groups=[list(range(NC))],
                ins=[ag_in.opt()], outs=[ag_out.opt()])
            if debug:
                agc = sb.tile([P, NMT, H], F32, tag="dbg_ag")
                nc.sync.dma_start(agc[:],
                                  ag_out[:].rearrange("(r p) d -> p r d", p=P))
                nc.sync.dma_start(
                    dbg_xagg.ap().rearrange("(r p) d -> p r d", p=P), agc[:])
                nc.sync.dma_start(dbg_zbias[:, :], zbias[:])

            # ================= Attention ===================================
            xaT_o = sb.tile([P, KH, P], F32R, tag="x_nT")
            for ko in range(KH):
                transpose_to(xaT_o[:, ko], xagg[:, ko * P:(ko + 1) * P])
            q_ps = ps_m.tile([P, H], F32, tag="mm256", space="PSUM")
            for ko in range(KH):
                nc.tensor.matmul(q_ps[:], xaT_o[:, ko], w_qkv_t[:, ko, 0:H],
                                 start=(ko == 0), stop=(ko == KH - 1))
            q_ln = sb.tile([P, H], F32, tag="x_ln")
            ln_rows(q_ps[:], q_ln[:])
            qT = keep.tile([P, 4, P], F32R)
            for h in range(NH):
                base = 64 * (h % 2)
                pt = ps_t.tile([P, P], F32, tag="tp", space="PSUM")
                nc.tensor.transpose(pt[:DHEAD, :],
                                    q_ln[:, h * DHEAD:(h + 1) * DHEAD], ident[:])
                nc.vector.tensor_copy(qT[base:base + DHEAD, h // 2],
                                      pt[:DHEAD, :])

            kT = keep.tile([P, 4, N], F32R)
            v_all = keep.tile([P, NMT, H], F32R)
            for m in range(NMT):
                agt = sb.tile([P, H], F32, tag="x_in")
                nc.sync.dma_start(agt[:], ag_out[m * P:(m + 1) * P, :])
                agT = sb.tile([P, KH, P], F32R, tag="x_nT")
                for ko in range(KH):
                    transpose_to(agT[:, ko], agt[:, ko * P:(ko + 1) * P])
                k_ps = ps_m.tile([P, H], F32, tag="mm256", space="PSUM")
                for ko in range(KH):
                    nc.tensor.matmul(k_ps[:], agT[:, ko], w_qkv_t[:, ko, H:2 * H],
                                     start=(ko == 0), stop=(ko == KH - 1))
                k_ln = sb.tile([P, H], F32, tag="x_n")
                ln_rows(k_ps[:], k_ln[:])
                for h in range(NH):
                    base = 64 * (h % 2)
                    pt = ps_t.tile([P, P], F32, tag="tp", space="PSUM")
                    nc.tensor.transpose(pt[:DHEAD, :],
                                        k_ln[:, h * DHEAD:(h + 1) * DHEAD],
                                        ident[:])
                    nc.vector.tensor_copy(
                        kT[base:base + DHEAD, h // 2, m * P:(m + 1) * P],
                        pt[:DHEAD, :])
                v_ps = ps_m.tile([P, H], F32, tag="mm256", space="PSUM")
                for ko in range(KH):
                    nc.tensor.matmul(v_ps[:], agT[:, ko],
                                     w_qkv_t[:, ko, 2 * H:3 * H],
                                     start=(ko == 0), stop=(ko == KH - 1))
                nc.vector.tensor_copy(v_all[:, m], v_ps[:])

            # mask: M' = sqrt(d) * (blockmask + zbias*(1-eye))
            mprime = keep.tile([P, N], F32)
            nc.vector.tensor_scalar(mprime[:], batch_bc[:], batch_own_t[:, 0:1],
                                    NEGBIG * SQD, op0=ALU.not_equal,
                                    op1=ALU.mult)
            dmask = sb.tile([P, N], F32, tag="s_full", bufs=1)
            nc.vector.tensor_scalar(dmask[:], iota[:], rowids[:, 0:1], SQD,
                                    op0=ALU.not_equal, op1=ALU.mult)
            nc.vector.tensor_mul(dmask[:], dmask[:], zbias[:])
            nc.vector.tensor_add(mprime[:], mprime[:], dmask[:])

            o_sb = sb.tile([P, H], F32, tag="o_sb")
            for h in range(NH):
                base = 64 * (h % 2)
                blk = h // 2
                s_sb = sb.tile([P, N], F32, tag="s_full", bufs=1)
                for half in range(2):
                    s_ps = ps_w.tile([P, 512], F32, tag="mm512", space="PSUM")
                    nc.tensor.matmul(s_ps[:], qT[base:base + DHEAD, blk],
                                     kT[base:base + DHEAD, blk,
                                        half * 512:(half + 1) * 512],
                                     start=True, stop=True)
                    nc.vector.tensor_add(
                        s_sb[:, half * 512:(half + 1) * 512], s_ps[:],
                        mprime[:, half * 512:(half + 1) * 512])
                p_sb = sb.tile([P, N], F32, tag="p_full", bufs=1)
                den = lnp.tile([P, 1], F32, tag="lnD")
                nc.scalar.activation(p_sb[:], s_sb[:], AF.Exp, scale=ISCALE,
                                     accum_out=den[:])
                rden = lnp.tile([P, 1], F32, tag="lnR")
                nc.vector.reciprocal(rden[:], den[:])
                pT = sb.tile([P, NMT, P], F32R, tag="pT", bufs=1)
                for m in range(NMT):
                    transpose_to(pT[:, m], p_sb[:, m * P:(m + 1) * P])
                o_ps = ps_av.tile([P, DHEAD], F32, tag="av", space="PSUM")
                for m in range(NMT):
                    nc.tensor.matmul(o_ps[:], pT[:, m],
                                     v_all[:, m, h * DHEAD:(h + 1) * DHEAD],
                                     start=(m == 0), stop=(m == NMT - 1))
                nc.scalar.activation(o_sb[:, h * DHEAD:(h + 1) * DHEAD], o_ps[:],
                                     AF.Copy, scale=rden[:, 0:1])

            oT = sb.tile([P, KH, P], F32R, tag="x_nT")
            for ko in range(KH):
                transpose_to(oT[:, ko], o_sb[:, ko * P:(ko + 1) * P])
            y_ps = ps_m.tile([P, H], F32, tag="mm256", space="PSUM")
            for ko in range(KH):
                nc.tensor.matmul(y_ps[:], oT[:, ko], w_out_t[:, ko],
                                 start=(ko == 0), stop=(ko == KH - 1))
            y_own = keep.tile([P, H], F32)
            nc.vector.tensor_copy(y_own[:], y_ps[:])
            nc.sync.dma_start(yag_in[:], y_own[:])
            nc.gpsimd.collective_compute(
                "AllGather", ALU.bypass, replica_groups=[list(range(NC))],
                ins=[yag_in.opt()], outs=[yg.opt()])
            if debug:
                nc.sync.dma_start(dbg_y[:, :], y_own[:])

            x2 = keep.tile([P, H], F32)
            nc.vector.tensor_mul(x2[:], y_own[:], own4[:, 0])
            xo = sb.tile([P, H], F32, tag="x_in")
            nc.sync.dma_start(xo[:], x_own[:, :])
            nc.vector.tensor_add(x2[:], x2[:], xo[:])

            # v = y @ W_e0 (all nodes, for E2 gathers)
            for m in range(NMT):
                ygt = sb.tile([P, H], F32, tag="x_in")
                nc.sync.dma_start(ygt[:], yg[m * P:(m + 1) * P, :])
                ygT = sb.tile([P, KH, P], F32R, tag="x_nT")
                for ko in range(KH):
                    transpose_to(ygT[:, ko], ygt[:, ko * P:(ko + 1) * P])
                v_ps2 = ps_m.tile([P, H], F32, tag="mm256", space="PSUM")
                for ko in range(KH):
                    nc.tensor.matmul(v_ps2[:], ygT[:, ko], w_e0_t[:, ko],
                                     start=(ko == 0), stop=(ko == KH - 1))
                vsb = sb.tile([P, H], F32, tag="u_out")
                nc.vector.tensor_copy(vsb[:], v_ps2[:])
                nc.sync.dma_start(v_d[m * P:(m + 1) * P, :], vsb[:])

            # ================= N3: node FFN (own rows) =====================
            w_ffn1_t = big1.tile([P, KH, 2 * FF], F32R, tag="wbig1")
            nc.sync.dma_start(
                w_ffn1_t[:], w_ffn1.ap().rearrange("(ko ki) n -> ki ko n", ki=P))
            w_ffn2_t = wts.tile([P, FF // P, H], F32R, tag="wbig2")
            nc.sync.dma_start(
                w_ffn2_t[:], w_ffn2.ap().rearrange("(ko ki) n -> ki ko n", ki=P))

            x2_ln = sb.tile([P, H], F32, tag="x_ln")
            ln_rows(x2[:], x2_ln[:])
            hn = sb.tile([P, H], F32, tag="x_n")
            nc.vector.tensor_mul(hn[:], x2_ln[:], own4[:, 2])
            nc.vector.tensor_add(hn[:], hn[:], own4[:, 1])
            hn2 = sb.tile([P, H], F32, tag="h_sum")
            ln_rows(hn[:], hn2[:])
            nc.vector.tensor_mul(hn2[:], hn2[:], g_ffn_bc[:])
            nc.vector.tensor_add(hn2[:], hn2[:], bt_ffn_bc[:])
            hnT = sb.tile([P, KH, P], F32R, tag="x_nT")
            for ko in range(KH):
                transpose_to(hnT[:, ko], hn2[:, ko * P:(ko + 1) * P])
            hT = sb.tile([P, FF // P, P], F32R, tag="n3hT", bufs=1)
            for mo in range(FF // P):
                a_ps = ps_m.tile([P, P], F32, tag="mm256", space="PSUM")
                for ko in range(KH):
                    nc.tensor.matmul(a_ps[:, :P],
                                     w_ffn1_t[:, ko, mo * P:(mo + 1) * P],
                                     hnT[:, ko], start=(ko == 0),
                                     stop=(ko == KH - 1))
                b_ps = ps_m.tile([P, P], F32, tag="mm256", space="PSUM")
                for ko in range(KH):
                    nc.tensor.matmul(b_ps[:, :P],
                                     w_ffn1_t[:, ko, FF + mo * P:FF + (mo + 1) * P],
                                     hnT[:, ko], start=(ko == 0),
                                     stop=(ko == KH - 1))
                sa = sb.tile([P, P], F32, tag="swi_a")
                nc.scalar.activation(sa[:], a_ps[:, :P], AF.Silu)
                nc.vector.tensor_tensor(hT[:, mo], sa[:], b_ps[:, :P], op=ALU.mult)
            f_ps = ps_m.tile([P, H], F32, tag="mm256", space="PSUM")
            for mo in range(FF // P):
                nc.tensor.matmul(f_ps[:], hT[:, mo], w_ffn2_t[:, mo],
                                 start=(mo == 0), stop=(mo == FF // P - 1))
            x3 = sb.tile([P, H], F32, tag="x_n")
            nc.vector.tensor_mul(x3[:], f_ps[:], own4[:, 3])
            nc.vector.tensor_add(x3[:], x3[:], x2[:])
            nc.sync.dma_start(x3_out[:, :], x3[:])

            # ================= E2: edge update + edge FFN ==================
            w_fe1e_t = big1.tile([P, KH, 2 * FF], F32R, tag="wbig1")
            nc.sync.dma_start(
                w_fe1e_t[:],
                w_ffn_e1.ap().rearrange("(ko ki) n -> ki ko n", ki=P))
            w_fe2e_t = wts.tile([P, FF // P, H], F32R, tag="wbig2")
            nc.sync.dma_start(
                w_fe2e_t[:],
                w_ffn_e2.ap().rearrange("(ko ki) n -> ki ko n", ki=P))
            # edge adaLN chunk cols 512:1536 (reuses the wae1 slot region? no:
            # separate tag sized for 1024 cols)
            w_ae2_t = wts.tile([P, KH, 1024], F32R, tag="wae")
            nc.sync.dma_start(
                w_ae2_t[:],
                w_adaln_e.ap()[:, 512:1536].rearrange("(ko ki) n -> ki ko n",
                                                      ki=P))
            b_e2 = bcast1d(sb, b_adaln_e, 1024, 512, "b_full",
                           plus1=(2 * H, 3 * H))
            b_e1m = None  # b_e1 no longer needed

            for bi in range(e_cap // 256):
                # process 256-edge half-blocks: 2 tiles each
                enT_blk = sb.tile([P, KH, 256], F32R, tag="e2enT")
                egml = sb.tile([P, 2, H], F32, tag="e2egml")
                edge_blk = sb.tile([P, 2, H], F32, tag="e2edge")
                for t in range(2):
                    it = bi * 2 + t
                    e0 = it * P
                    te = sb.tile([P, H], F32, tag="t_in")
                    nc.sync.dma_start(te[:], te_loc[e0:e0 + P, :])
                    ste = sb.tile([P, H], F32, tag="t_silu")
                    nc.scalar.activation(ste[:], te[:], AF.Silu)
                    steT = sb.tile([P, KH, P], F32R, tag="t_siluT")
                    for ko in range(KH):
                        transpose_to(steT[:, ko], ste[:, ko * P:(ko + 1) * P])
                    me2 = sb.tile([P, 4, H], F32, tag="e2me", bufs=1)
                    for half in range(2):
                        mp = ps_w.tile([P, 512], F32, tag="mm512", space="PSUM")
                        for ko in range(KH):
                            nc.tensor.matmul(
                                mp[:], steT[:, ko],
                                w_ae2_t[:, ko, half * 512:(half + 1) * 512],
                                start=(ko == 0), stop=(ko == KH - 1))
                        nc.vector.tensor_add(
                            me2[:, 2 * half:2 * half + 2].rearrange(
                                "p a b -> p (a b)"),
                            mp[:], b_e2[:, half * 512:(half + 1) * 512])
                    nc.gpsimd.tensor_copy(egml[:, t], me2[:, 3])

                    sidx = sb.tile([P, 1], I32, tag="si")
                    nc.sync.dma_start(sidx[:], src_g[e0:e0 + P, :])
                    tidx = sb.tile([P, 1], I32, tag="ti")
                    nc.sync.dma_start(tidx[:], tgt_g[e0:e0 + P, :])
                    vs = sb.tile([P, H], F32, tag="g_us")
                    nc.gpsimd.indirect_dma_start(
                        out=vs[:], out_offset=None, in_=v_d[:, :],
                        in_offset=bass.IndirectOffsetOnAxis(ap=sidx[:, 0:1],
                                                            axis=0))
                    vt = sb.tile([P, H], F32, tag="g_ut")
                    nc.gpsimd.indirect_dma_start(
                        out=vt[:], out_offset=None, in_=v_d[:, :],
                        in_offset=bass.IndirectOffsetOnAxis(ap=tidx[:, 0:1],
                                                            axis=0))
                    dsum = sb.tile([P, H], F32, tag="h_sum")
                    nc.gpsimd.tensor_tensor(dsum[:], vs[:], vt[:], op=ALU.add)
                    ea = sb.tile([P, H], F32, tag="x_in")
                    nc.sync.dma_start(ea[:], ea_loc[e0:e0 + P, :])
                    nc.vector.tensor_mul(edge_blk[:, t], dsum[:], me2[:, 0])
                    nc.vector.tensor_add(edge_blk[:, t], edge_blk[:, t], ea[:])

                    eT = sb.tile([P, KH, P], F32R, tag="x_nT")
                    for ko in range(KH):
                        transpose_to(eT[:, ko],
                                     edge_blk[:, t, ko * P:(ko + 1) * P])
                    dt_ = sb.tile([P, DV], F32, tag="d_in")
                    nc.sync.dma_start(dt_[:], dist_loc[e0:e0 + P, :])
                    dT = sb.tile([P, P], F32R, tag="d_T")
                    transpose_to(dT[:], dt_[:])
                    ei_ps = ps_m.tile([P, H], F32, tag="mm256", space="PSUM")
                    for ko in range(KH):
                        nc.tensor.matmul(ei_ps[:], eT[:, ko], w_e1_t[:, ko],
                                         start=(ko == 0), stop=False)
                    nc.tensor.matmul(ei_ps[:], dT[:], w_e1_t[:, 2],
                                     start=False, stop=True)
                    ei_ln = sb.tile([P, H], F32, tag="x_ln")
                    ln_rows(ei_ps[:], ei_ln[:])
                    enm = sb.tile([P, H], F32, tag="x_n")
                    nc.vector.tensor_mul(enm[:], ei_ln[:], me2[:, 2])
                    nc.vector.tensor_add(enm[:], enm[:], me2[:, 1])
                    en2 = sb.tile([P, H], F32, tag="h_fe")
                    ln_rows(enm[:], en2[:])
                    nc.gpsimd.tensor_tensor(en2[:], en2[:], g_ffn_e_bc[:],
                                            op=ALU.mult)
                    nc.gpsimd.tensor_tensor(en2[:], en2[:], bt_ffn_e_bc[:],
                                            op=ALU.add)
                    for ko in range(KH):
                        transpose_to(enT_blk[:, ko, t * P:(t + 1) * P],
                                     en2[:, ko * P:(ko + 1) * P])

                hTe = sb.tile([P, FF // P, 256], F32R, tag="e2hT", bufs=1)
                for mo in range(FF // P):
                    a_ps = ps_w.tile([P, 512], F32, tag="mm512", space="PSUM")
                    for ko in range(KH):
                        nc.tensor.matmul(a_ps[:, :256],
                                         w_fe1e_t[:, ko, mo * P:(mo + 1) * P],
                                         enT_blk[:, ko], start=(ko == 0),
                                         stop=(ko == KH - 1))
                    b_ps = ps_w.tile([P, 512], F32, tag="mm512", space="PSUM")
                    for ko in range(KH):
                        nc.tensor.matmul(
                            b_ps[:, :256],
                            w_fe1e_t[:, ko, FF + mo * P:FF + (mo + 1) * P],
                            enT_blk[:, ko], start=(ko == 0), stop=(ko == KH - 1))
                    sa = sb.tile([P, 256], F32, tag="swi_a")
                    nc.scalar.activation(sa[:], a_ps[:, :256], AF.Silu)
                    nc.vector.tensor_tensor(hTe[:, mo], sa[:], b_ps[:, :256],
                                            op=ALU.mult)
                for t in range(2):
                    it = bi * 2 + t
                    e0 = it * P
                    f_ps = ps_m.tile([P, H], F32, tag="mm256", space="PSUM")
                    for mo in range(FF // P):
                        nc.tensor.matmul(f_ps[:], hTe[:, mo, t * P:(t + 1) * P],
                                         w_fe2e_t[:, mo],
                                         start=(mo == 0),
                                         stop=(mo == FF // P - 1))
                    eo = sb.tile([P, H], F32, tag="u_out")
                    nc.vector.tensor_mul(eo[:], f_ps[:], egml[:, t])
                    nc.vector.tensor_add(eo[:], eo[:], edge_blk[:, t])
                    nc.sync.dma_start(edge_out[e0:e0 + P, :], eo[:])

    nc.compile()
    return nc


# ======================= host driver =======================

def _prep(inputs):
    ins = {k: np.asarray(v) for k, v in inputs.items()}
    f = lambda k: np.ascontiguousarray(ins[k], dtype=np.float32)
    src = ins["edge_index"][0].astype(np.int64)
    tgt = ins["edge_index"][1].astype(np.int64)
    E = src.shape[0]
    owner = src // NLOC
    perm = np.argsort(owner, kind="stable")
    counts = np.bincount(owner, minlength=NC)
    e_cap = max(512, int(math.ceil(counts.max() / 512.0)) * 512)
    cnt_node = np.bincount(src, minlength=N).astype(np.float32)
    denom = np.maximum(cnt_node, 1.0)

    x = f("x"); th = f("t_emb_h"); ea = f("edge_attr"); te = f("t_emb_e")
    dist = f("dist"); Z = f("Z")
    batch = f("batch")

    starts = np.zeros(NC + 1, np.int64)
    starts[1:] = np.cumsum(counts)
    in_maps = []
    for c in range(NC):
        sel = perm[starts[c]:starts[c + 1]]
        n_c = sel.shape[0]

        def pad2(a, w):
            out = np.zeros((e_cap, w), np.float32)
            out[:n_c] = a[sel]
            return out

        src_c = np.zeros((e_cap, 1), np.int32)
        src_c[:n_c, 0] = src[sel]
        tgt_c = np.zeros((e_cap, 1), np.int32)
        tgt_c[:n_c, 0] = tgt[sel]
        a_tc = np.zeros((e_cap, NLOC), np.float32)
        loc = (src[sel] - c * NLOC).astype(np.int64)
        a_tc[np.arange(n_c), loc] = 1.0 / denom[src[sel]]
        rows = slice(c * NLOC, (c + 1) * NLOC)
        in_maps.append(dict(
            x_full=x, th_full=th,
            x_own=np.ascontiguousarray(x[rows]),
            th_own=np.ascontiguousarray(th[rows]),
            batch_f=batch,
            batch_own=np.ascontiguousarray(batch[rows])[:, None],
            row_ids=np.arange(c * NLOC, (c + 1) * NLOC,
                              dtype=np.float32)[:, None],
            z_loc=np.ascontiguousarray(Z[rows]),
            ea_loc=pad2(ea, H), te_loc=pad2(te, H), dist_loc=pad2(dist, DV),
            src_g=src_c, tgt_g=tgt_c, a_t=a_tc,
            w_adaln=f("W_adaLN"), b_adaln=f("b_adaLN"),
            w_adaln_e=f("W_adaLN_e"), b_adaln_e=f("b_adaLN_e"),
            w_fe1=f("W_fe1"), b_fe1=f("b_fe1"),
            w_fe2=f("W_fe2"), b_fe2=f("b_fe2"),
            w_qkv=f("W_qkv"), w_out=f("W_out"), w_pb=f("W_pb"),
            w_e0=f("W_e0"), w_e1=f("W_e1"),
            g_ffn=f("g_ffn"), bt_ffn=f("bt_ffn"),
            w_ffn1=f("W_ffn1"), w_ffn2=f("W_ffn2"),
            g_ffn_e=f("g_ffn_e"), bt_ffn_e=f("bt_ffn_e"),
            w_ffn_e1=f("W_ffn_e1"), w_ffn_e2=f("W_ffn_e2"),
        ))
    return in_maps, perm, counts, e_cap, E


_CACHE = {}


def kernel(**inputs):
    in_maps, perm, counts, e_cap, E = _prep(inputs)
    if e_cap not in _CACHE:
        _CACHE[e_cap] = build_kernel(e_cap)
    nc = _CACHE[e_cap]
    res = run_bass_kernel_spmd(nc, in_maps, core_ids=list(range(NC))).results
    x3 = np.concatenate([res[c]["x3_out"] for c in range(NC)], axis=0)
    starts = np.zeros(NC + 1, np.int64)
    starts[1:] = np.cumsum(counts)
    edge_full = np.empty((E, H), np.float32)
    for c in range(NC):
        sel = perm[starts[c]:starts[c + 1]]
        edge_full[sel] = res[c]["edge_out"][:sel.shape[0]]
    return x3, edge_full
